# revision 56
# baseline (speedup 1.0000x reference)
"""2-layer GAT (PyG GATConv semantics) on 8 Trainium2 NeuronCores.

Layout strategy:
- Nodes are packed into "quads" of 128 packed rows; core c owns quads
  [c*qper, (c+1)*qper) -> all per-quad DRAM addresses are core-invariant
  (SPMD) while per-core index/dstlocal input arrays carry the graph.
- Feature tables are bf16 with 256-byte rows (the dma_gather minimum):
    table1 row: [h1 (64) | asrc1 (2) | adst1 (2) | pad]      (128 bf16)
    table2 row: [h2 * a_src2 (128)]                          (128 bf16)
  Layer 2 pre-scales h2 by a_src2 per feature so the per-edge source
  logit is a plain row-sum of the gathered row (recovered after the
  segment-sum by multiplying with 1/a_src2); adst2 never needs gathering
  (destinations are core-local) and lives in a local [shard,1] array.
- Per quad, incoming edges are split by source bank (bank = 2 cores'
  shards <= 32768 rows, the int16 reach of dma_gather). Each (quad, bank)
  sub-tile packs its edges into 128 single-destination rows (a node's
  edges may split across rows; softmax numerator/denominator are sums, so
  partials recombine exactly). Pad slots point at the bank's last row
  (every core's last shard row is a pad row written with asrc=-300 before
  the AllGather, so exp(leaky_relu(e)) == 0 and Shared-table single-writer
  holds).
- Per sub-tile: e = prelu_0.2(asrc + adst_row) (ACT, per-partition bias),
  p = exp(e) with fp32 accum -> softmax-denominator partial, msg = p*h
  (DVE bf16 broadcast mult in place), row-reduce (DVE, bf16 partials).
  Small one-hot bf16 matmuls (PE) map bank partials back to node order,
  accumulating in PSUM; normalization divides by the summed denominator.
- Dense phases (x@W1ext, h1@W2ext) are sharded across cores in bf16;
  results are shared with two AllGather collectives into Shared-space
  tables.
"""

import os

import numpy as np

import concourse.bacc as bacc
import concourse.mybir as mybir
import concourse.tile as tile
from concourse.bass_utils import run_bass_kernel_spmd

F32 = mybir.dt.float32
BF16 = mybir.dt.bfloat16
I16 = mybir.dt.int16
AF = mybir.ActivationFunctionType
ALU = mybir.AluOpType

NCORES = 8
NEG = 0.2
QR = 128              # rows per quad

FIN = 128
H1, FH = 2, 32
D1 = H1 * FH          # 64
FOUT = 128
RW = 128              # table row width in bf16 elems (256 B)
GRP = int(os.environ.get("GAT_GRP", "4"))  # quads per gather group

NPBF = None           # set in preprocess: numpy bf16 dtype


def _npbf():
    return mybir.dt.np(BF16)


def _wrap_idx(idx):
    """int16 index list -> [128, n/16]: wrapped in 16 partitions, replicated
    across the 8 Q7 cores (HW-verified layout)."""
    n = idx.shape[0]
    assert n % 16 == 0
    blk = idx.reshape(n // 16, 16).T.astype(np.int16)
    return np.tile(blk, (8, 1))


def _groups(qper):
    return [list(range(g, min(g + GRP, qper))) for g in range(0, qper, GRP)]


def preprocess(x, edge_index, W1, att_src1, att_dst1, b1, W2, att_src2,
               att_dst2, b2):
    bf = _npbf()
    N = x.shape[0]
    src = np.concatenate([edge_index[0], np.arange(N, dtype=np.int64)])
    dst = np.concatenate([edge_index[1], np.arange(N, dtype=np.int64)])
    src = src.astype(np.int64)
    dst = dst.astype(np.int64)

    NPQ = int(os.environ.get("GAT_NPQ", "112"))  # nodes per quad (rows: 128)
    nquads = (N + NPQ - 1) // NPQ
    nquads = ((nquads + NCORES - 1) // NCORES) * NCORES
    NP = nquads * QR
    qper = nquads // NCORES
    shard = NP // NCORES
    BANK = 2 * shard                # bank = 2 cores' shards (int16 reach)
    assert BANK <= 32768
    nbanks = NP // BANK
    pad_rows = [(b + 1) * BANK - 1 for b in range(nbanks)]
    # every core's last shard row is a pad row (written pre-AllGather so the
    # Shared table needs no post-collective fixup)
    pad_set = {(c + 1) * shard - 1 for c in range(NCORES)}
    assert nquads * NPQ >= N

    deg = np.bincount(dst, minlength=N).astype(np.int64)

    # --- greedy LPT node->quad packing (equal edges per quad) ---
    import heapq
    qcap = np.full(nquads, NPQ, dtype=np.int64)
    qload = np.zeros(nquads, dtype=np.int64)
    qcount = np.zeros(nquads, dtype=np.int64)
    heap = [(0, 0, q) for q in range(nquads)]
    heapq.heapify(heap)
    node_quad = np.empty(N, dtype=np.int64)
    for n in np.argsort(-deg, kind="stable"):
        while True:
            _, _, q = heapq.heappop(heap)
            if qcount[q] < qcap[q]:
                break
        node_quad[n] = q
        qcount[q] += 1
        qload[q] += deg[n]
        heapq.heappush(heap, (qload[q], qcount[q], q))

    packed_of_node = np.empty(N, dtype=np.int64)
    nodes_by_quad = [[] for _ in range(nquads)]
    for n in range(N):
        nodes_by_quad[node_quad[n]].append(n)
    for q in range(nquads):
        s = q * QR
        for n in nodes_by_quad[q]:
            while s in pad_set:
                s += 1
            packed_of_node[n] = s
            s += 1

    psrc = packed_of_node[src]
    pdst = packed_of_node[dst]
    equad = pdst // QR
    ebank = psrc // BANK

    # --- bucket edges by (quad, bank) ---
    okey = equad * nbanks + ebank
    order = np.argsort(okey, kind="stable")
    sorted_src = psrc[order]
    sorted_dst = pdst[order]
    counts = np.bincount(okey, minlength=nquads * nbanks)
    starts = np.concatenate([[0], np.cumsum(counts)])

    # per-(quad,bank) segment sizes -> minimal feasible ncols
    def min_ncols(seg_sizes, total):
        nc0 = max(1, (total + QR - 1) // QR)
        while True:
            if sum((s + nc0 - 1) // nc0 for s in seg_sizes) <= QR:
                return nc0
            nc0 += 1

    C = np.ones((qper, nbanks), dtype=np.int64)
    for q in range(nquads):
        for b in range(nbanks):
            s0, s1 = starts[q * nbanks + b], starts[q * nbanks + b + 1]
            edst_b = pdst[order[s0:s1]]
            if s1 > s0:
                _, segc = np.unique(edst_b, return_counts=True)
                segs = segc.tolist()
            else:
                segs = []
            k = q % qper
            C[k, b] = max(C[k, b], min_ncols(segs, s1 - s0))

    groups = _groups(qper)

    # --- per-core arrays ---
    in_maps = []
    xT = np.zeros((FIN, NP), dtype=np.float32)
    xT[:, packed_of_node] = np.asarray(x, dtype=np.float32).T
    xT = xT.astype(bf)

    W1 = np.asarray(W1, dtype=np.float32)
    W2 = np.asarray(W2, dtype=np.float32)
    a_s1 = np.asarray(att_src1, dtype=np.float32)
    a_d1 = np.asarray(att_dst1, dtype=np.float32)
    a_s2 = np.asarray(att_src2, dtype=np.float32)
    a_d2 = np.asarray(att_dst2, dtype=np.float32)
    W1a_s = np.stack([W1[:, h * FH:(h + 1) * FH] @ a_s1[h] for h in range(H1)], 1)
    W1a_d = np.stack([W1[:, h * FH:(h + 1) * FH] @ a_d1[h] for h in range(H1)], 1)
    Wext1 = np.concatenate([W1, W1a_s, W1a_d], axis=1)          # [FIN, 68]
    # layer 2: h2 pre-scaled by a_src2 per feature; adst2 as an extra column
    Wext2 = np.concatenate([W2 * a_s2[0][None, :],
                            (W2 @ a_d2[0])[:, None]], axis=1)   # [D1, 129]
    inv_a2 = (1.0 / a_s2[0]).astype(np.float32)                 # [FOUT]
    b1e = np.zeros((1, D1 + H1), dtype=np.float32)
    b1e[0, :D1] = b1
    b2e = np.zeros((1, FOUT + 1), dtype=np.float32)
    b2e[0, :FOUT] = b2
    padrow1 = np.zeros((1, RW), dtype=np.float32)
    padrow1[0, D1:D1 + H1] = -300.0
    padrow2 = np.zeros((1, RW), dtype=np.float32)
    padrow2[0, 0] = -300.0

    const = {
        "Wext1": Wext1.astype(bf), "Wext2": Wext2.astype(bf),
        "b1e": b1e.astype(bf), "b2e": b2e.astype(bf),
        "iota_rep": np.tile(np.arange(QR, dtype=np.float32)[None],
                            (QR, 1)).astype(bf),
        "iota_col": np.arange(QR, dtype=np.float32)[:, None],
        "ones_row": np.ones((1, QR), dtype=np.float32).astype(bf),
        "ident": np.eye(QR, dtype=np.float32).astype(bf),
        "padrow1": padrow1.astype(bf), "padrow2": padrow2.astype(bf),
        "invA": np.tile(inv_a2[None], (QR, 1)),                 # [128,128] f32
    }

    for c in range(NCORES):
        idx_parts = []
        dl_col = np.zeros((qper, QR, nbanks), dtype=np.float32)
        dl_row = np.zeros((qper, nbanks, QR), dtype=np.float32)
        for grp in groups:
            for b in range(nbanks):
                for k in grp:
                    q = c * qper + k
                    s0, s1 = starts[q * nbanks + b], starts[q * nbanks + b + 1]
                    esrc = sorted_src[s0:s1]
                    edst = sorted_dst[s0:s1]
                    ncols = C[k, b]
                    # pack edges into 128 single-node rows (splitting allowed)
                    rows_src = np.full((QR, ncols), pad_rows[b] - b * BANK,
                                       dtype=np.int64)
                    rownode = np.zeros(QR, dtype=np.int64)
                    o2 = np.argsort(edst, kind="stable")
                    esrc, edst = esrc[o2], edst[o2]
                    bnd = np.concatenate(
                        [[0], np.where(np.diff(edst) != 0)[0] + 1, [len(edst)]])
                    segs = [(edst[bnd[i]], bnd[i], bnd[i + 1])
                            for i in range(len(bnd) - 1)] if len(edst) else []
                    segs.sort(key=lambda t: -(t[2] - t[1]))
                    r = 0
                    for nd, a0, a1 in segs:
                        pos = a0
                        while pos < a1:
                            assert r < QR
                            take = min(ncols, a1 - pos)
                            rows_src[r, :take] = esrc[pos:pos + take] - b * BANK
                            rownode[r] = nd % QR
                            pos += take
                            r += 1
                    idxs = rows_src.T.reshape(-1)   # column-major slots
                    idx_parts.append(_wrap_idx(idxs))
                    dl_col[k, :, b] = rownode
                    dl_row[k, b, :] = rownode
        im = dict(const)
        im["xT"] = np.ascontiguousarray(xT[:, c * shard:(c + 1) * shard])
        im["dl_col"] = dl_col
        im["dl_row"] = dl_row.astype(bf)
        im["gidx"] = np.concatenate(idx_parts, axis=1)
        in_maps.append(im)

    meta = {
        "N": N, "NP": NP, "qper": qper, "shard": shard, "nbanks": nbanks,
        "C": C, "pad_rows": pad_rows, "packed_of_node": packed_of_node,
        "bank_rows": [BANK] * nbanks, "BANK": BANK,
    }
    return in_maps, meta


def build(nc, meta, reps=1):
    qper, nbanks = meta["qper"], meta["nbanks"]
    C, NP, shard = meta["C"], meta["NP"], meta["shard"]
    groups = _groups(qper)
    n_idx_cols = int(C.sum()) * 8       # wrapped width units (n/16)

    xT_in = nc.dram_tensor("xT", [FIN, shard], BF16, kind="ExternalInput")
    Wext1_in = nc.dram_tensor("Wext1", [FIN, D1 + 2 * H1], BF16,
                              kind="ExternalInput")
    Wext2_in = nc.dram_tensor("Wext2", [D1, FOUT + 1], BF16,
                              kind="ExternalInput")
    b1e_in = nc.dram_tensor("b1e", [1, D1 + H1], BF16, kind="ExternalInput")
    b2e_in = nc.dram_tensor("b2e", [1, FOUT + 1], BF16, kind="ExternalInput")
    irep_in = nc.dram_tensor("iota_rep", [QR, QR], BF16, kind="ExternalInput")
    icol_in = nc.dram_tensor("iota_col", [QR, 1], F32, kind="ExternalInput")
    ones_in = nc.dram_tensor("ones_row", [1, QR], BF16, kind="ExternalInput")
    ident_in = nc.dram_tensor("ident", [QR, QR], BF16, kind="ExternalInput")
    pr1_in = nc.dram_tensor("padrow1", [1, RW], BF16, kind="ExternalInput")
    pr2_in = nc.dram_tensor("padrow2", [1, RW], BF16, kind="ExternalInput")
    invA_in = nc.dram_tensor("invA", [QR, QR], F32, kind="ExternalInput")
    dlc_in = nc.dram_tensor("dl_col", [qper, QR, nbanks], F32,
                            kind="ExternalInput")
    dlr_in = nc.dram_tensor("dl_row", [qper, nbanks, QR], BF16,
                            kind="ExternalInput")
    gidx_in = nc.dram_tensor("gidx", [QR, n_idx_cols], I16,
                             kind="ExternalInput")
    out_ext = nc.dram_tensor("out", [shard, FOUT], F32, kind="ExternalOutput")
    dbg = {}
    if os.environ.get("GAT_DEBUG"):
        dbg["bA"] = nc.dram_tensor("dbg_bA", [shard, RW], BF16,
                                   kind="ExternalOutput")
        dbg["bB"] = nc.dram_tensor("dbg_bB", [shard, RW], BF16,
                                   kind="ExternalOutput")
        dbg["ad"] = nc.dram_tensor("dbg_ad", [shard, 1], BF16,
                                   kind="ExternalOutput")

    with tile.TileContext(nc) as tc:
        with tc.tile_pool(name="dram", bufs=1, space="DRAM") as dr:
            with tc.tile_pool(name="const", bufs=1) as cst:
                Wext1_t = cst.tile([FIN, D1 + 2 * H1], BF16)
                Wext2_t = cst.tile([D1, FOUT + 1], BF16)
                b1e_t = cst.tile([1, D1 + H1], BF16)
                b2e_t = cst.tile([1, FOUT + 1], BF16)
                irep_t = cst.tile([QR, QR], BF16)
                icol_t = cst.tile([QR, 1], F32)
                ones_t = cst.tile([1, QR], BF16)
                ident_t = cst.tile([QR, QR], BF16)
                pr1_t = cst.tile([1, RW], BF16)
                pr2_t = cst.tile([1, RW], BF16)
                invA_t = cst.tile([QR, QR], F32)
                for t, s in [(Wext1_t, Wext1_in), (Wext2_t, Wext2_in),
                             (b1e_t, b1e_in), (b2e_t, b2e_in),
                             (irep_t, irep_in), (icol_t, icol_in),
                             (ones_t, ones_in), (ident_t, ident_in),
                             (pr1_t, pr1_in), (pr2_t, pr2_in),
                             (invA_t, invA_in)]:
                    nc.sync.dma_start(t[:], s[:])

                consts = dict(
                    Wext1_t=Wext1_t, Wext2_t=Wext2_t, b1e_t=b1e_t,
                    b2e_t=b2e_t, irep_t=irep_t, icol_t=icol_t, ones_t=ones_t,
                    ident_t=ident_t, pr1_t=pr1_t, pr2_t=pr2_t, invA_t=invA_t)
                for _ in range(reps):
                    _emit_body(nc, tc, dr, meta, groups, xT_in, consts,
                               dlc_in, dlr_in, gidx_in, out_ext, dbg)
    return nc


def _emit_body(nc, tc, dr, meta, groups, xT_in, consts,
               dlc_in, dlr_in, gidx_in, out_ext, dbg=None):
    qper, nbanks = meta["qper"], meta["nbanks"]
    C, NP, shard = meta["C"], meta["NP"], meta["shard"]
    tspace = ("Local" if (os.environ.get("GAT_NOSHARED")
                          or os.environ.get("GAT_SIM1")) else "Shared")
    bounceA = dr.tile([shard, RW], BF16, name="bounceA")
    table1 = dr.tile([NP, RW], BF16, addr_space=tspace, name="table1")
    bounceB = dr.tile([shard, RW], BF16, name="bounceB")
    table2 = dr.tile([NP, RW], BF16, addr_space=tspace, name="table2")
    adstB = dr.tile([shard, 1], BF16, name="adstB")
    cc = dict(consts)
    cc["adstB"] = adstB

    # ---------- phase A: bounceA = [x@W1 | asrc1 | adst1] -------
    PAB = 2                                  # quads per phase-A batch
    with (
        tc.tile_pool(name="pa", bufs=3) as pa,
        tc.tile_pool(name="pa_ps", bufs=2 * PAB, space="PSUM") as pa_ps,
    ):
        for k0 in range(0, shard // QR, PAB):
            pb = min(PAB, shard // QR - k0)
            xTt = pa.tile([FIN, pb * QR], BF16, tag="xT")
            nc.sync.dma_start(xTt[:], xT_in[:, k0 * QR:(k0 + pb) * QR])
            st = pa.tile([QR, pb, D1 + 2 * H1], BF16, tag="stA")
            for g in range(pb):
                ps = pa_ps.tile([QR, D1 + 2 * H1], F32, tag="psA")
                nc.tensor.matmul(ps[:], xTt[:, g * QR:(g + 1) * QR],
                                 cc["Wext1_t"][:], start=True, stop=True)
                nc.scalar.copy(st[:, g, :], ps[:])
            nc.sync.dma_start(
                bounceA[k0 * QR:(k0 + pb) * QR, 0:D1 + 2 * H1].rearrange(
                    "(g p) c -> p g c", g=pb), st[:])

    # pad row: every core's last shard row, written pre-collective
    nc.sync.dma_start(bounceA[shard - 1:shard, :], cc["pr1_t"][:])
    if os.environ.get("GAT_SIM1"):
        for c8 in range(NCORES):
            nc.sync.dma_start(table1[c8 * shard:(c8 + 1) * shard, :],
                              bounceA[:])
    elif not os.environ.get("GAT_SKIP_AG"):
        nc.gpsimd.collective_compute(
            "AllGather", ALU.bypass,
            replica_groups=[list(range(NCORES))],
            ins=[bounceA[:].opt()], outs=[table1[:].opt()])

    # ---------- layer 1 + fused phase A' ------------------------
    if os.environ.get("GAT_SKIP_L1"):
        nc.sync.dma_start(bounceB[:, :], bounceA[:, :])
    else:
        _emit_layer(nc, tc, meta, groups, layer=1,
                    table=table1, feat=D1, heads=H1,
                    adst_src=bounceA, adst_off=D1 + H1, cc=cc,
                    dlc_in=dlc_in, dlr_in=dlr_in, gidx_in=gidx_in,
                    bounceB=bounceB, out_ext=None)

    nc.sync.dma_start(bounceB[shard - 1:shard, :], cc["pr2_t"][:])
    if os.environ.get("GAT_SIM1"):
        for c8 in range(NCORES):
            nc.sync.dma_start(table2[c8 * shard:(c8 + 1) * shard, :],
                              bounceB[:])
    elif not os.environ.get("GAT_SKIP_AG"):
        nc.gpsimd.collective_compute(
            "AllGather", ALU.bypass,
            replica_groups=[list(range(NCORES))],
            ins=[bounceB[:].opt()], outs=[table2[:].opt()])

    # ---------- layer 2 ----------------------------------------
    if os.environ.get("GAT_SKIP_L2"):
        nc.sync.dma_start(out_ext[:, 0:RW // 2], bounceB[:, 0:RW // 2])
        nc.sync.dma_start(out_ext[:, RW // 2:RW], bounceB[:, RW // 2:RW])
    else:
        _emit_layer(nc, tc, meta, groups, layer=2,
                    table=table2, feat=FOUT, heads=1,
                    adst_src=adstB, adst_off=0, cc=cc,
                    dlc_in=dlc_in, dlr_in=dlr_in, gidx_in=gidx_in,
                    bounceB=None, out_ext=out_ext)
    if dbg:
        nc.sync.dma_start(dbg["bA"][:], bounceA[:])
        nc.sync.dma_start(dbg["bB"][:], bounceB[:])
        nc.sync.dma_start(dbg["ad"][:], adstB[:])


def _emit_layer(nc, tc, meta, groups, layer, table, feat, heads,
                adst_src, adst_off, cc, dlc_in, dlr_in, gidx_in,
                bounceB, out_ext):
    qper, nbanks = meta["qper"], meta["nbanks"]
    C, bank_rows = meta["C"], meta["bank_rows"]
    BANK_ = meta["BANK"]
    ocols = feat + heads                     # [reduced | s]
    hw = feat // heads                       # per-head width
    Cmax = int(C.max())

    with (
        tc.tile_pool(name=f"gL{layer}",
                     bufs=int(os.environ.get("GAT_GBUF", "3"))) as gp,
        tc.tile_pool(name=f"wL{layer}", bufs=3) as wp,
        tc.tile_pool(name=f"oL{layer}", bufs=2 * nbanks + 2) as op,
        tc.tile_pool(name=f"repL{layer}", bufs=2, space="PSUM") as rep_ps,
        tc.tile_pool(name=f"adL{layer}", bufs=2, space="PSUM") as ad_ps,
        tc.tile_pool(name=f"cmbL{layer}", bufs=2, space="PSUM") as cmb_ps,
        tc.tile_pool(name=f"auxL{layer}", bufs=1, space="PSUM") as aux_ps,
    ):
        idx_col = 0
        for grp in groups:
            ng = len(grp)
            k0 = grp[0]
            # gather group slab
            slab_cols = sum(int(C[k][b]) for k in grp for b in range(nbanks))
            G = gp.tile([QR, slab_cols, RW], BF16, tag="G")
            grp_icols = slab_cols * QR // 16
            it = wp.tile([QR, grp_icols], I16, tag="idx")
            nc.sync.dma_start(it[:], gidx_in[:, idx_col:idx_col + grp_icols])
            off_map = {}
            off = 0
            i_off = 0
            for b in range(nbanks):
                bcols = sum(int(C[k][b]) for k in grp)
                nidx = bcols * QR
                if not os.environ.get("GAT_SKIP_GATHER"):
                    nc.gpsimd.dma_gather(
                        out_ap=G[:, off:off + bcols, :],
                        in_ap=table[b * BANK_:b * BANK_ + bank_rows[b], :],
                        idxs_ap=it[:, i_off:i_off + nidx // 16],
                        num_idxs=nidx, num_idxs_reg=nidx, elem_size=RW,
                        single_packet=bool(os.environ.get("GAT_SP")),
                        queue_num=b % 4)
                o2 = off
                for k in grp:
                    off_map[(k, b)] = o2
                    o2 += int(C[k][b])
                off += bcols
                i_off += nidx // 16
            idx_col += grp_icols

            # batched per-group loads
            dlc_t = wp.tile([QR, ng, nbanks], F32, tag="dlc")
            nc.sync.dma_start(
                dlc_t[:], dlc_in[k0:k0 + ng].rearrange("g p b -> p g b"))
            dlr_t = wp.tile([1, ng * nbanks * QR], BF16, tag="dlr")
            nc.sync.dma_start(
                dlr_t[:], dlr_in[k0:k0 + ng].rearrange("g b p -> (g b p)")[
                    None, :])
            adq_t = wp.tile([QR, ng, heads], BF16, tag="adq")
            nc.sync.dma_start(
                adq_t[:],
                adst_src[k0 * QR:(k0 + ng) * QR,
                         adst_off:adst_off + heads].rearrange(
                    "(g p) h -> p g h", g=ng))
            if layer == 1:
                st2g = wp.tile([QR, ng, FOUT], BF16, tag="st2g")
                adbg = wp.tile([QR, ng], BF16, tag="adbg")
            else:
                otg = wp.tile([QR, ng, FOUT], F32, tag="otg")

            for gi, k in enumerate(grp):
                psq = cmb_ps.tile([QR, ocols], F32, tag="psq")
                col_scatter = os.environ.get("GAT_SCATTER") != "reduce"
                if col_scatter:
                    # bias first: initializes the full psq region once, so
                    # every later matmul accumulates (start=False)
                    bias_t = cc["b1e_t"] if layer == 1 else cc["b2e_t"]
                    nc.tensor.matmul(psq[:], cc["ones_t"][:], bias_t[:],
                                     start=True, stop=False)
                for b in range(nbanks):
                    cb = int(C[k][b])
                    c0 = off_map[(k, b)]
                    # one-hot rows->nodes
                    P1 = op.tile([QR, QR], BF16, tag="P1")
                    nc.vector.tensor_scalar(
                        out=P1[:], in0=cc["irep_t"][:],
                        scalar1=dlc_t[:, gi, b:b + 1],
                        scalar2=None, op0=ALU.is_equal)
                    if os.environ.get("GAT_P2") == "rep":
                        rep = rep_ps.tile([QR, QR], F32, tag="rep")
                        do = (gi * nbanks + b) * QR
                        nc.tensor.matmul(
                            rep[:], cc["ones_t"][:], dlr_t[0:1, do:do + QR],
                            start=True, stop=True)
                        P2 = op.tile([QR, QR], BF16, tag="P2")
                        nc.vector.tensor_scalar(
                            out=P2[:], in0=rep[:], scalar1=cc["icol_t"][:],
                            scalar2=None, op0=ALU.is_equal)
                    else:
                        # P2 = P1^T via the PE (saves a DVE is_equal)
                        repT = rep_ps.tile([QR, QR], BF16, tag="rep")
                        nc.tensor.transpose(repT[:], P1[:], cc["ident_t"][:])
                        P2 = op.tile([QR, QR], BF16, tag="P2")
                        nc.scalar.copy(P2[:], repT[:])
                    adp = ad_ps.tile([QR, heads], F32, tag="adp")
                    nc.tensor.matmul(adp[:], P2[:], adq_t[:, gi, :],
                                     start=True, stop=True)
                    ads = wp.tile([QR, heads], F32, tag="ads")
                    nc.scalar.copy(ads[:], adp[:])

                    lg = wp.tile([QR, heads, Cmax], F32, tag="lg")
                    Pb = wp.tile([QR, heads, Cmax], BF16, tag="Pb")
                    Sacc = wp.tile([QR, heads], F32, tag="Sacc")
                    if layer == 2:
                        # slot logit = row-sum of prescaled h2 (a_src2 folded)
                        ss = wp.tile([QR, Cmax], F32, tag="ss")
                        nc.vector.tensor_reduce(
                            out=ss[:, 0:cb],
                            in_=G[:, c0:c0 + cb, 0:feat],
                            axis=mybir.AxisListType.X, op=ALU.add)
                    for h in range(heads):
                        lg_in = (ss[:, 0:cb] if layer == 2
                                 else G[:, c0:c0 + cb, feat + h])
                        nc.scalar.activation(
                            lg[:, h, 0:cb], lg_in,
                            AF.Prelu, bias=ads[:, h:h + 1], alpha=NEG)
                        nc.scalar.activation(
                            Pb[:, h, 0:cb], lg[:, h, 0:cb], AF.Exp,
                            accum_out=Sacc[:, h:h + 1])
                    # weighted messages: one fused mult across all heads
                    nc.vector.tensor_tensor(
                        out=G[:, c0:c0 + cb, 0:feat].rearrange(
                            "p j (h w) -> p j h w", h=heads),
                        in0=G[:, c0:c0 + cb, 0:feat].rearrange(
                            "p j (h w) -> p j h w", h=heads),
                        in1=Pb[:, :, 0:cb].rearrange(
                            "p h j -> p j h")[:, :, :, None].broadcast_to(
                            [QR, cb, heads, hw]),
                        op=ALU.mult)
                    if not col_scatter:
                        O = op.tile([QR, ocols], BF16, tag="O")
                        with nc.allow_low_precision(
                                reason="bf16 single-dest row partials"):
                            nc.vector.tensor_reduce(
                                out=O[:, 0:feat],
                                in_=G[:, c0:c0 + cb, 0:feat].rearrange(
                                    "p j f -> p f j"),
                                axis=mybir.AxisListType.X, op=ALU.add)
                        nc.scalar.copy(O[:, feat:feat + heads], Sacc[:])
                        nc.tensor.matmul(psq[:], P1[:], O[:],
                                         start=(b == 0), stop=False)
                    else:
                        # scatter rows->nodes directly on the PE: one
                        # stationary P1, cb weighted-message columns + the
                        # denominator, accumulated across banks in PSUM
                        Sb = wp.tile([QR, heads], BF16, tag="Sb")
                        nc.scalar.copy(Sb[:], Sacc[:])
                        for j in range(cb):
                            nc.tensor.matmul(
                                psq[:, 0:feat], P1[:], G[:, c0 + j, 0:feat],
                                start=False, stop=False)
                        nc.tensor.matmul(psq[:, feat:feat + heads], P1[:],
                                         Sb[:], start=False,
                                         stop=(b == nbanks - 1))

                if not col_scatter:
                    bias_t = cc["b1e_t"] if layer == 1 else cc["b2e_t"]
                    nc.tensor.matmul(psq[:], cc["ones_t"][:], bias_t[:],
                                     start=False, stop=True)

                rs = wp.tile([QR, heads], F32, tag="rs")
                nc.vector.tensor_scalar(
                    out=rs[:], in0=psq[:, feat:feat + heads],
                    scalar1=1e-30, scalar2=None, op0=ALU.max)
                nc.vector.reciprocal(rs[:], rs[:])
                if layer == 1:
                    ot = wp.tile([QR, feat], F32, tag="ot")
                    nc.vector.tensor_tensor(
                        out=ot[:].rearrange("p (h f) -> p h f", h=heads),
                        in0=psq[:, 0:feat].rearrange("p (h f) -> p h f",
                                                     h=heads),
                        in1=rs[:, :, None].broadcast_to([QR, heads, hw]),
                        op=ALU.mult)
                    otb = wp.tile([QR, D1], BF16, tag="otb")
                    nc.scalar.activation(otb[:], ot[:], AF.Relu)
                    psT = aux_ps.tile([D1, QR], BF16, tag="psT")
                    nc.tensor.transpose(psT[:], otb[:], cc["ident_t"][:])
                    h1T = wp.tile([D1, QR], BF16, tag="h1T")
                    nc.scalar.copy(h1T[:], psT[:])
                    ps2 = aux_ps.tile([QR, FOUT + 1], F32, tag="ps2")
                    nc.tensor.matmul(ps2[:], h1T[:], cc["Wext2_t"][:],
                                     start=True, stop=True)
                    nc.vector.tensor_copy(st2g[:, gi, :], ps2[:, 0:FOUT])
                    nc.scalar.copy(adbg[:, gi:gi + 1], ps2[:, FOUT:FOUT + 1])
                else:
                    # normalize and recover numerator (1/a_src2 per feature)
                    nc.vector.scalar_tensor_tensor(
                        out=otg[:, gi, :], in0=psq[:, 0:feat],
                        scalar=rs[:, 0:1], in1=cc["invA_t"][:],
                        op0=ALU.mult, op1=ALU.mult)

            # batched per-group stores
            if layer == 1:
                nc.sync.dma_start(
                    bounceB[k0 * QR:(k0 + ng) * QR, 0:FOUT].rearrange(
                        "(g p) c -> p g c", g=ng), st2g[:])
                nc.sync.dma_start(
                    cc["adstB"][k0 * QR:(k0 + ng) * QR, :].rearrange(
                        "(g p) c -> p (g c)", g=ng), adbg[:])
            else:
                nc.sync.dma_start(
                    out_ext[k0 * QR:(k0 + ng) * QR, :].rearrange(
                        "(g p) c -> p g c", g=ng), otg[:])


def kernel(x, edge_index, W1, att_src1, att_dst1, b1, W2, att_src2, att_dst2,
           b2):
    import time as _time
    _t = _time.time()
    in_maps, meta = preprocess(x, edge_index, W1, att_src1, att_dst1, b1,
                               W2, att_src2, att_dst2, b2)
    print(f"[kernel] preprocess {_time.time() - _t:.1f}s", flush=True)
    _t = _time.time()
    nc = bacc.Bacc("TRN2", num_devices=NCORES, target_bir_lowering=False,
                   num_swdge_queues=4)
    build(nc, meta)
    print(f"[kernel] build {_time.time() - _t:.1f}s "
          f"({len(nc.inst_map)} inst)", flush=True)
    _t = _time.time()
    nc.compile()
    print(f"[kernel] bacc compile {_time.time() - _t:.1f}s", flush=True)
    _t = _time.time()
    trace = bool(os.environ.get("GAT_TRACE"))
    r = run_bass_kernel_spmd(nc, in_maps, list(range(NCORES)), trace=trace)
    print(f"[kernel] hw run {_time.time() - _t:.1f}s", flush=True)
    global _last_hw_ns
    _last_hw_ns = (_time.time() - _t) * 1e9
    if trace and r.exec_time_ns is not None:
        print(f"HW exec time: {r.exec_time_ns} ns", flush=True)
        _last_hw_ns = r.exec_time_ns
    global _last_results, _last_meta, _last_inmaps
    _last_results, _last_meta, _last_inmaps = r, meta, in_maps
    shard = meta["shard"]
    full = np.concatenate([r.results[c]["out"] for c in range(NCORES)], axis=0)
    out = full[meta["packed_of_node"]]
    return np.ascontiguousarray(out.astype(np.float32))
